# revision 24
# baseline (speedup 1.0000x reference)
"""Trainium2 Bass kernel for LongRangeAttention.

Block-local MHA (8 heads, segment=64) + pooled global MHA (4 heads) over
segment means, broadcast back and summed.

Sharding: 8 cores; core c handles batch b=c//2, token half h=c%2 (2048
tokens = 32 segments). Each core gets its half feature-major ([D, 2048]:
xl) plus the other half (xr) for the segment means; global attention over
segment means is permutation-equivariant, so each core computes the pooled
attention redundantly with its own segments in rows 0:32.

All compute in bf16 (f32 PSUM accumulation). Local tokens processed in 4
chunks of 512 tokens (4 slices of 128 = 2 segments each). The segment mask
is folded into the score matmul as a rank-3 update; softmax skips the
max-subtraction (scores are O(1) by construction); softmax scales are
folded into the q-side weights host-side, and the 1/64 mean scaling into
the global weights.
"""

import numpy as np

B, T, D = 4, 4096, 1024
SEG = 64
N_CORES = 8
TL = T // 2          # tokens per core
NSEG = T // SEG      # segments per batch element (64)
HL, HDL = 8, 128     # local heads
HG, HDG = 4, 256     # global heads
CH = 512             # tokens per chunk
NC_CH = TL // CH     # 4 chunks
MASK_VAL = 30000.0

_CACHE = {}


def _split_drain_tile_context():
    """TileContext whose kernel-tail drain spreads its sem waits across SP
    nops — the walrus build here rejects >2 sync waits on CTRL instrs."""
    from bass_rust import N_PROCS
    from concourse import tile as tile_mod
    from concourse.vector_clock import ScopedClock, VectorClock

    class SplitDrainTileContext(tile_mod.TileContext):
        def _drain_and_barrier(self, tick_clock, wait_clock):
            gc = tick_clock.global_clock
            for p in range(N_PROCS):
                if gc[p] > 0:
                    vc = VectorClock(
                        [gc[q] if q == p else 0 for q in range(N_PROCS)]
                    )
                    nop = self.nc.sync.nop(hint=f"drain_split_{p}", nofuse=True)
                    wait_clock.add_sem_waits(nop.ins, ScopedClock({None: vc}))
            # The SP nops above precede the drain in program order, so all
            # sems have reached the global clock before it executes.
            self.nc.sync.drain()
            self.nc.all_engine_barrier()
            popped = self.nc._tile_sem_poison_stack.pop()
            assert popped is self._sem_poison
            self.nc.clear_and_free_semaphores(list(self.sems.allocated().values()))
            self.nc.all_engine_barrier()

    return SplitDrainTileContext


def _fixup_waits(nc, max_waits=2):
    """This walrus build rejects instructions with >2 sync waits. Hoist the
    excess onto same-engine nops inserted just before the instruction —
    program order on the engine preserves the gating semantics."""
    import concourse.mybir as mybir

    ctr = [0]
    for f in nc.m.functions:
        for bb in f.blocks:
            new_insts = []
            for inst in bb.instructions:
                max_waits = 1
                si = inst.sync_info
                waits = list(si.on_wait) if si and si.on_wait else []
                if len(waits) > max_waits:
                    excess, keep = waits[:-max_waits], waits[-max_waits:]
                    for i in range(0, len(excess), max_waits):
                        nop = mybir.InstNoOp(name=f"waitnop{ctr[0]}", ins=[], outs=[])
                        ctr[0] += 1
                        nop.engine = inst.engine
                        nop.sync_info = mybir.SyncInfo(
                            on_wait=excess[i:i + max_waits], on_update=[]
                        )
                        new_insts.append(nop)
                    inst.sync_info = mybir.SyncInfo(
                        on_wait=keep, on_update=si.on_update
                    )
                new_insts.append(inst)
            if len(new_insts) != len(bb.instructions):
                try:
                    bb.instructions = new_insts
                except Exception:
                    bb.instructions[:] = new_insts
    return nc


def _build_nc():
    import concourse.bass as bass
    import concourse.mybir as mybir

    f32 = mybir.dt.float32
    bf16 = mybir.dt.bfloat16
    X = mybir.AxisListType.X
    Exp = mybir.ActivationFunctionType.Exp
    TC = _split_drain_tile_context()

    nc = bass.Bass()
    dp = nc.declare_dram_parameter
    xl = dp("xl", [D, TL], bf16, isOutput=False)
    xr = dp("xr", [D, TL], bf16, isOutput=False)
    wqk = dp("wqk", [D, 2 * D], bf16, isOutput=False)
    wv = dp("wv", [D, D], bf16, isOutput=False)
    wo = dp("wo", [D, D], bf16, isOutput=False)
    wgqk = dp("wgqk", [D, 2 * D], bf16, isOutput=False)
    wgv = dp("wgv", [D, D], bf16, isOutput=False)
    wgo = dp("wgo", [D, D], bf16, isOutput=False)
    bqk = dp("bqk", [2 * D, 1], f32, isOutput=False)
    bgqkr = dp("bgqkr", [1, 2 * D], bf16, isOutput=False)
    bv = dp("bv", [1, D], bf16, isOutput=False)
    bgv = dp("bgv", [1, D], bf16, isOutput=False)
    boc = dp("boc", [1, D], bf16, isOutput=False)
    ident = dp("ident", [128, 128], bf16, isOutput=False)
    maskl = dp("maskl", [3, 128], bf16, isOutput=False)
    maskr = dp("maskr", [3, 128], bf16, isOutput=False)
    bcast = dp("bcast", [SEG, TL], bf16, isOutput=False)
    ones = dp("ones", [1, 128], bf16, isOutput=False)
    out = dp("out", [TL, D], bf16, isOutput=True)

    with TC(nc) as tc:
        with (
            tc.tile_pool(name="persist", bufs=1) as pp,
            tc.tile_pool(name="xrp", bufs=2) as xrp,
            tc.tile_pool(name="gwp", bufs=2) as gwp,
            tc.tile_pool(name="gvp", bufs=2) as gvp,
            tc.tile_pool(name="Pp", bufs=2) as Pp,
            tc.tile_pool(name="PTp", bufs=2) as PTp,
            tc.tile_pool(name="aop", bufs=3) as aop,
            tc.tile_pool(name="sump", bufs=3) as sump,
            tc.tile_pool(name="osbp", bufs=2) as osbp,
            tc.tile_pool(name="ps", bufs=1, space="PSUM") as psp,
        ):
            # ---- persistent SBUF tiles ----
            wqk_sb = [pp.tile([128, 2 * D], bf16, tag=f"wqk{d}", name=f"wqk{d}") for d in range(8)]
            wv_sb = [pp.tile([128, D], bf16, tag=f"wv{d}", name=f"wv{d}") for d in range(8)]
            wo_sb = [pp.tile([128, D], bf16, tag=f"wo{d}", name=f"wo{d}") for d in range(8)]
            xl_sb = [pp.tile([128, TL], bf16, tag=f"xl{d}", name=f"xl{d}") for d in range(8)]
            means = [pp.tile([128, NSEG], bf16, tag=f"mn{d}", name=f"mn{d}") for d in range(8)]
            # 2 rotating sets of qk tiles (16 j-slices) and v tiles (4 slices)
            qk_sb = [
                [pp.tile([128, CH], bf16, tag=f"qk{st}_{j}", name=f"qk{st}_{j}")
                 for j in range(16)]
                for st in range(2)
            ]
            v_sb = [
                [pp.tile([128, D], bf16, tag=f"v{st}_{s}", name=f"v{st}_{s}")
                 for s in range(4)]
                for st in range(2)
            ]
            ao0_sb = [pp.tile([128, CH], bf16, tag=f"ao0_{g}", name=f"ao0_{g}")
                      for g in range(8)]
            qkg_sb = [pp.tile([128, 512], bf16, tag=f"qkg{i}", name=f"qkg{i}") for i in range(2)]
            vg_sb = pp.tile([SEG, D], bf16, tag="vg", name="vg")
            og_sb = pp.tile([128, 512], bf16, tag="og", name="og")
            outg_sb = pp.tile([SEG, D], bf16, tag="outg", name="outg")
            Pg_sb = pp.tile([SEG, 256], bf16, tag="Pg", name="Pg")
            PgT_sb = pp.tile([SEG, 256], bf16, tag="PgT", name="PgT")
            gsum_sb = pp.tile([SEG, 4], f32, tag="gsum", name="gsum")
            grr_sb = pp.tile([SEG, 4], f32, tag="grr", name="grr")

            ident_sb = pp.tile([128, 128], bf16, tag="ident", name="ident")
            maskl_sb = pp.tile([3, 128], bf16, tag="maskl", name="maskl")
            maskr_sb = pp.tile([3, 128], bf16, tag="maskr", name="maskr")
            bcast_sb = pp.tile([SEG, TL], bf16, tag="bcast", name="bcast")
            ones_sb = pp.tile([1, 128], bf16, tag="ones", name="ones")
            bqk_sb = pp.tile([128, 16], f32, tag="bqk", name="bqk")
            bgqkr_sb = pp.tile([1, 2 * D], bf16, tag="bgqkr", name="bgqkr")
            bv_sb = pp.tile([1, D], bf16, tag="bv", name="bv")
            bgv_sb = pp.tile([1, D], bf16, tag="bgv", name="bgv")
            boc_sb = pp.tile([1, D], bf16, tag="boc", name="boc")

            # ---- input DMAs: sync ring carries the PE-critical stream ----
            for d in range(8):
                nc.sync.dma_start(out=wqk_sb[d][:], in_=wqk[d * 128:(d + 1) * 128, :])
            for d in range(8):
                nc.sync.dma_start(
                    out=xl_sb[d][:, 0:CH], in_=xl[d * 128:(d + 1) * 128, 0:CH]
                )
            nc.sync.dma_start(out=ident_sb[:], in_=ident[:])
            nc.sync.dma_start(out=maskl_sb[:], in_=maskl[:])
            nc.sync.dma_start(out=maskr_sb[:], in_=maskr[:])
            nc.sync.dma_start(out=ones_sb[:], in_=ones[:])
            nc.sync.dma_start(
                out=bqk_sb[:], in_=bqk.rearrange("(j p) o -> p (j o)", p=128)
            )
            nc.sync.dma_start(out=bv_sb[:], in_=bv[:])
            for d in range(8):
                nc.sync.dma_start(out=wv_sb[d][:], in_=wv[d * 128:(d + 1) * 128, :])
            for d in range(8):
                nc.sync.dma_start(
                    out=xl_sb[d][:, CH:TL], in_=xl[d * 128:(d + 1) * 128, CH:TL]
                )
            nc.sync.dma_start(out=bcast_sb[:], in_=bcast[:])
            nc.sync.dma_start(out=boc_sb[:], in_=boc[:])
            # global qk weights stream on the otherwise-idle sync-ring tail;
            # first half prefetched ahead of wo so wo isn't gated on the
            # pool-slot waits of the later tiles.
            wgqk_tiles = []
            for d in range(4):
                wt = gwp.tile([128, 2 * D], bf16, tag="gqk", name=f"wgqk{d}")
                nc.sync.dma_start(out=wt[:], in_=wgqk[d * 128:(d + 1) * 128, :])
                wgqk_tiles.append(wt)
            for d in range(8):
                nc.sync.dma_start(out=wo_sb[d][:], in_=wo[d * 128:(d + 1) * 128, :])
            for d in range(4, 8):
                wt = gwp.tile([128, 2 * D], bf16, tag="gqk", name=f"wgqk{d}")
                nc.sync.dma_start(out=wt[:], in_=wgqk[d * 128:(d + 1) * 128, :])
                wgqk_tiles.append(wt)

            # ---- scalar ring: remote-half x (for means) + global v/o prep.
            # xr streams through a small pool; the DVE means reduces are
            # interleaved into the emission flow below so they never block
            # the chunk-0 qk copies on the in-order vector stream.
            xr_tiles = []

            def emit_xr_dmas():
                for d in range(2):
                    t_ = xrp.tile([128, TL], bf16, tag="xr", name=f"xr{d}")
                    nc.scalar.dma_start(out=t_[:], in_=xr[d * 128:(d + 1) * 128, :])
                    xr_tiles.append(t_)
                nc.scalar.dma_start(out=bgqkr_sb[:], in_=bgqkr[:])
                nc.scalar.dma_start(out=bgv_sb[:], in_=bgv[:])
                for d in range(2, 8):
                    t_ = xrp.tile([128, TL], bf16, tag="xr", name=f"xr{d}")
                    nc.scalar.dma_start(out=t_[:], in_=xr[d * 128:(d + 1) * 128, :])
                    xr_tiles.append(t_)

            def emit_means_xr(ds):
                with nc.allow_low_precision(reason="bf16 segment sums"):
                    for d in ds:
                        nc.vector.reduce_sum(
                            out=means[d][:, 32:64],
                            in_=xr_tiles[d].rearrange("p (s t) -> p s t", t=SEG),
                            axis=X,
                        )

            def emit_means_xl(ds):
                with nc.allow_low_precision(reason="bf16 segment sums"):
                    for d in ds:
                        nc.vector.reduce_sum(
                            out=means[d][:, 0:32],
                            in_=xl_sb[d].rearrange("p (s t) -> p s t", t=SEG),
                            axis=X,
                        )

            # ---- local projections for one chunk ----
            def emit_qk_tile(c, j):
                """qk projection for j-slice of chunk c into qk set c%2."""
                st = c % 2
                ps = psp.tile([128, CH], f32, tag="qk", name=f"psqk{c}_{j}", bufs=2)
                for d in range(8):
                    nc.tensor.matmul(
                        ps[:],
                        lhsT=wqk_sb[d][:, j * 128:(j + 1) * 128],
                        rhs=xl_sb[d][:, c * CH:(c + 1) * CH],
                        start=(d == 0),
                        stop=(d == 7),
                    )
                nc.vector.tensor_scalar_add(
                    qk_sb[st][j][:], ps[:], bqk_sb[:, j:j + 1]
                )

            def emit_v_slice(c, s):
                """v projection for 128-token slice s of chunk c."""
                st = c % 2
                tok0 = c * CH + s * 128
                for nb in range(2):
                    ps = psp.tile([128, 512], f32, tag="v", name=f"psv{c}_{s}_{nb}", bufs=2)
                    for d in range(8):
                        nc.tensor.matmul(
                            ps[:],
                            lhsT=xl_sb[d][:, tok0:tok0 + 128],
                            rhs=wv_sb[d][:, nb * 512:(nb + 1) * 512],
                            start=(d == 0),
                            stop=False,
                        )
                    nc.tensor.matmul(
                        ps[:],
                        lhsT=ones_sb[0:1, :],
                        rhs=bv_sb[0:1, nb * 512:(nb + 1) * 512],
                        start=False,
                        stop=True,
                    )
                    nc.vector.tensor_copy(
                        v_sb[st][s][:, nb * 512:(nb + 1) * 512], ps[:]
                    )

            # ---- attention pieces (group g = 4 heads of one slice) ----
            def emit_scores(c, s, g):
                """scores+mask for heads 4g..4g+3 of slice s -> one PSUM bank;
                exp on ACT; per-head sums + recip on DVE; normalize on Pool.
                Returns (P, rr) tiles."""
                st = c % 2
                col0 = s * 128
                ps = psp.tile([128, 512], f32, tag="scpt", name=f"pss{c}_{s}_{g}", bufs=2)
                for hh in range(4):
                    h = g * 4 + hh
                    nc.tensor.matmul(
                        ps[:, hh * 128:(hh + 1) * 128],
                        lhsT=qk_sb[st][h][:, col0:col0 + 128],
                        rhs=qk_sb[st][8 + h][:, col0:col0 + 128],
                        start=(hh == 0),
                        stop=False,
                        skip_group_check=True,
                    )
                for hh in range(4):
                    nc.tensor.matmul(
                        ps[:, hh * 128:(hh + 1) * 128],
                        lhsT=maskl_sb[:],
                        rhs=maskr_sb[:],
                        start=False,
                        stop=(hh == 3),
                        skip_group_check=True,
                    )
                P = Pp.tile([128, 512], bf16, tag="P", name=f"P{c}_{s}_{g}")
                nc.scalar.activation(P[:], ps[:], Exp)
                ssum = sump.tile([128, 4], f32, tag="ssum", name=f"ss{c}_{s}_{g}")
                nc.vector.reduce_sum(
                    out=ssum[:],
                    in_=P.rearrange("p (h k) -> p h k", k=128),
                    axis=X,
                )
                rr = sump.tile([128, 4], f32, tag="rr", name=f"rr{c}_{s}_{g}")
                nc.vector.reciprocal(rr[:], ssum[:])
                for hh in range(4):
                    nc.gpsimd.tensor_scalar_mul(
                        P[:, hh * 128:(hh + 1) * 128],
                        P[:, hh * 128:(hh + 1) * 128],
                        rr[:, hh:hh + 1],
                    )
                return P

            def emit_attn(c, s, g, P, ao):
                """transpose P + attention output for group g of slice s."""
                st = c % 2
                ps_t = psp.tile([128, 1024], bf16, tag="scpt", name=f"pst{c}_{s}_{g}", bufs=2)
                for hh in range(4):
                    nc.tensor.transpose(
                        ps_t[:, hh * 128:(hh + 1) * 128],
                        P[:, hh * 128:(hh + 1) * 128],
                        ident_sb[:],
                    )
                PT = PTp.tile([128, 512], bf16, tag="PT", name=f"PT{c}_{s}_{g}")
                nc.scalar.copy(PT[:], ps_t[:, 0:512])
                ps_a = psp.tile([128, 512], f32, tag="atop", name=f"psa{c}_{s}_{g}", bufs=2)
                for hh in range(4):
                    h = g * 4 + hh
                    nc.tensor.matmul(
                        ps_a[:, hh * 128:(hh + 1) * 128],
                        lhsT=v_sb[st][s][:, h * 128:(h + 1) * 128],
                        rhs=PT[:, hh * 128:(hh + 1) * 128],
                        start=(hh == 0),
                        stop=(hh == 3),
                        skip_group_check=True,
                    )
                nc.vector.tensor_copy(ao[:], ps_a[:])

            def emit_outproj(c, s, aos):
                """output projection + global broadcast + bias for slice s."""
                tok0 = c * CH + s * 128
                osb = osbp.tile([128, D], bf16, tag="osb", name=f"osb{c}_{s}")
                for nb in range(2):
                    ps_o = psp.tile([128, 512], f32, tag="atop", name=f"pso{c}_{s}_{nb}", bufs=2)
                    for h in range(8):
                        nc.tensor.matmul(
                            ps_o[:],
                            lhsT=aos[h // 4][:, (h % 4) * 128:(h % 4 + 1) * 128],
                            rhs=wo_sb[h][:, nb * 512:(nb + 1) * 512],
                            start=(h == 0),
                            stop=False,
                        )
                    nc.tensor.matmul(
                        ps_o[:],
                        lhsT=bcast_sb[:, tok0:tok0 + 128],
                        rhs=outg_sb[:, nb * 512:(nb + 1) * 512],
                        start=False,
                        stop=False,
                    )
                    nc.tensor.matmul(
                        ps_o[:],
                        lhsT=ones_sb[0:1, :],
                        rhs=boc_sb[0:1, nb * 512:(nb + 1) * 512],
                        start=False,
                        stop=True,
                    )
                    nc.vector.tensor_copy(osb[:, nb * 512:(nb + 1) * 512], ps_o[:])
                nc.gpsimd.dma_start(
                    out=out[tok0:tok0 + 128, :], in_=osb[:]
                )

            # ---- global path over segment means ----
            def emit_global():
                # qkg: j-slices packed 8 per PSUM bank; d-outer so the
                # streamed wgqk tiles are fully consumed on one visit.
                ps_qg = [
                    psp.tile([128, 512], f32, tag="qk", name=f"psqkg{h}", bufs=2)
                    for h in range(2)
                ]
                for d in range(8):
                    for j in range(16):
                        half, jj = divmod(j, 8)
                        nc.tensor.matmul(
                            ps_qg[half][:, jj * 64:(jj + 1) * 64],
                            lhsT=wgqk_tiles[d][:, j * 128:(j + 1) * 128],
                            rhs=means[d][:, :],
                            start=(d == 0 and jj == 0),
                            stop=False,
                            skip_group_check=True,
                        )
                for j in range(16):
                    half, jj = divmod(j, 8)
                    nc.tensor.matmul(
                        ps_qg[half][:, jj * 64:(jj + 1) * 64],
                        lhsT=bgqkr_sb[0:1, j * 128:(j + 1) * 128],
                        rhs=ones_sb[0:1, 0:64],
                        start=False,
                        stop=(jj == 7),
                        skip_group_check=True,
                    )
                for half in range(2):
                    nc.vector.tensor_copy(qkg_sb[half][:], ps_qg[half][:])
                # vg: d-outer with both nb-half PSUMs live; wgv streamed.
                ps_vg = [
                    psp.tile([SEG, 512], f32, tag="v", name=f"psvg{nb}", bufs=2)
                    for nb in range(2)
                ]
                for d in range(8):
                    wt = gvp.tile([128, D], bf16, tag="gv", name=f"gv{d}")
                    nc.scalar.dma_start(out=wt[:], in_=wgv[d * 128:(d + 1) * 128, :])
                    for nb in range(2):
                        nc.tensor.matmul(
                            ps_vg[nb][:],
                            lhsT=means[d][:, :],
                            rhs=wt[:, nb * 512:(nb + 1) * 512],
                            start=(d == 0),
                            stop=False,
                        )
                for nb in range(2):
                    nc.tensor.matmul(
                        ps_vg[nb][:],
                        lhsT=ones_sb[0:1, 0:SEG],
                        rhs=bgv_sb[0:1, nb * 512:(nb + 1) * 512],
                        start=False,
                        stop=True,
                    )
                    nc.vector.tensor_copy(
                        vg_sb[:, nb * 512:(nb + 1) * 512], ps_vg[nb][:]
                    )
                # scores: 4 heads x [64,64] packed into one bank
                ps_s = psp.tile([SEG, 512], f32, tag="scpt", name="psgs", bufs=2)
                for h in range(4):
                    for cpart in range(2):
                        j = h * 2 + cpart
                        nc.tensor.matmul(
                            ps_s[:, h * 64:(h + 1) * 64],
                            lhsT=qkg_sb[0][:, j * 64:(j + 1) * 64],
                            rhs=qkg_sb[1][:, j * 64:(j + 1) * 64],
                            start=(h == 0 and cpart == 0),
                            stop=(h == 3 and cpart == 1),
                            skip_group_check=True,
                        )
                nc.scalar.activation(Pg_sb[:], ps_s[:, 0:256], Exp)
                nc.vector.reduce_sum(
                    out=gsum_sb[:],
                    in_=Pg_sb.rearrange("p (h k) -> p h k", k=SEG),
                    axis=X,
                )
                nc.vector.reciprocal(grr_sb[:], gsum_sb[:])
                for h in range(4):
                    nc.gpsimd.tensor_scalar_mul(
                        Pg_sb[:, h * 64:(h + 1) * 64],
                        Pg_sb[:, h * 64:(h + 1) * 64],
                        grr_sb[:, h:h + 1],
                    )
                ps_t = psp.tile([SEG, 1024], bf16, tag="scpt", name="psgt", bufs=2)
                for h in range(4):
                    nc.tensor.transpose(
                        ps_t[:, h * 64:(h + 1) * 64],
                        Pg_sb[:, h * 64:(h + 1) * 64],
                        ident_sb[0:SEG, 0:SEG],
                    )
                nc.scalar.copy(PgT_sb[:], ps_t[:, 0:256])
                # og: 8 j-slices packed into one bank [128, 512]
                ps_og = psp.tile([128, 512], f32, tag="atop", name="psog", bufs=2)
                for j in range(8):
                    h = j // 2
                    nc.tensor.matmul(
                        ps_og[:, j * 64:(j + 1) * 64],
                        lhsT=vg_sb[:, j * 128:(j + 1) * 128],
                        rhs=PgT_sb[:, h * 64:(h + 1) * 64],
                        start=(j == 0),
                        stop=(j == 7),
                        skip_group_check=True,
                    )
                nc.vector.tensor_copy(og_sb[:], ps_og[:])
                # outg: j-outer with both nb-half PSUMs live; wgo streamed.
                ps_g = [
                    psp.tile([SEG, 512], f32, tag="v", name=f"psoutg{nb}", bufs=2)
                    for nb in range(2)
                ]
                for j in range(8):
                    wt = gvp.tile([128, D], bf16, tag="gv", name=f"go{j}")
                    nc.scalar.dma_start(out=wt[:], in_=wgo[j * 128:(j + 1) * 128, :])
                    for nb in range(2):
                        nc.tensor.matmul(
                            ps_g[nb][:],
                            lhsT=og_sb[:, j * 64:(j + 1) * 64],
                            rhs=wt[:, nb * 512:(nb + 1) * 512],
                            start=(j == 0),
                            stop=(j == 7),
                        )
                for nb in range(2):
                    nc.vector.tensor_copy(
                        outg_sb[:, nb * 512:(nb + 1) * 512], ps_g[nb][:]
                    )

            # ================= emission =================
            emit_xr_dmas()

            # chunk 0 projections, means reduces slotted into the DVE stream
            for j in range(8):
                emit_qk_tile(0, j)
            emit_means_xr(range(0, 4))
            for j in range(8, 16):
                emit_qk_tile(0, j)
            emit_means_xr(range(4, 8))
            for s in range(4):
                emit_v_slice(0, s)
                emit_means_xl(range(2 * s, 2 * s + 2))

            # chunk 0 attention, interleaved with chunk 1 projections;
            # outproj(0) deferred until after the global phase.
            for s in range(4):
                P0 = emit_scores(0, s, 0)
                P1 = emit_scores(0, s, 1)
                emit_qk_tile(1, 4 * s)
                emit_qk_tile(1, 4 * s + 1)
                emit_attn(0, s, 0, P0, ao0_sb[2 * s])
                emit_qk_tile(1, 4 * s + 2)
                emit_qk_tile(1, 4 * s + 3)
                emit_attn(0, s, 1, P1, ao0_sb[2 * s + 1])
                emit_v_slice(1, s)

            emit_global()

            # chunks 1..3: attention + inline outproj, interleaved with next
            # chunk's projections (chunk 3 uses the deferred outproj(0)).
            for c in range(1, 4):
                for s in range(4):
                    P0 = emit_scores(c, s, 0)
                    P1 = emit_scores(c, s, 1)
                    if c < 3:
                        emit_qk_tile(c + 1, 4 * s)
                        emit_qk_tile(c + 1, 4 * s + 1)
                    else:
                        emit_outproj(0, s, [ao0_sb[2 * s], ao0_sb[2 * s + 1]])
                    a0 = aop.tile([128, 512], bf16, tag="ao", name=f"ao{c}_{s}_0")
                    emit_attn(c, s, 0, P0, a0)
                    if c < 3:
                        emit_qk_tile(c + 1, 4 * s + 2)
                        emit_qk_tile(c + 1, 4 * s + 3)
                    a1 = aop.tile([128, 512], bf16, tag="ao", name=f"ao{c}_{s}_1")
                    emit_attn(c, s, 1, P1, a1)
                    emit_outproj(c, s, [a0, a1])
                    if c < 3:
                        emit_v_slice(c + 1, s)
    return _fixup_waits(nc)


def _mask_arrays():
    import ml_dtypes

    bf = ml_dtypes.bfloat16
    b1 = np.zeros(128, np.float32)
    b1[:64] = 1.0
    b2 = 1.0 - b1
    onesr = np.ones(128, np.float32)
    maskl = np.stack([onesr, b1, b2]).astype(bf)          # [3,128] lhsT
    maskr = np.stack([-MASK_VAL * onesr, MASK_VAL * b1, MASK_VAL * b2]).astype(bf)
    return maskl, maskr


def _bcast():
    m = np.zeros((SEG, TL), np.float32)
    for t in range(TL):
        m[t // SEG, t] = 1.0
    return m


def _shard_inputs(inputs):
    """Build the 8 per-core input maps from the full problem inputs."""
    import ml_dtypes

    f = np.float32
    bf = ml_dtypes.bfloat16
    x = np.asarray(inputs["x"], f)
    w_in_l = np.asarray(inputs["w_in_local"], f)
    b_in_l = np.asarray(inputs["b_in_local"], f)
    w_out_l = np.asarray(inputs["w_out_local"], f)
    b_out_l = np.asarray(inputs["b_out_local"], f)
    w_in_g = np.asarray(inputs["w_in_global"], f)
    b_in_g = np.asarray(inputs["b_in_global"], f)
    w_out_g = np.asarray(inputs["w_out_global"], f)
    b_out_g = np.asarray(inputs["b_out_global"], f)

    scl_l = 1.0 / np.sqrt(HDL)
    scl_g = 1.0 / np.sqrt(HDG)
    # local qk weights, q-half scaled by softmax scale
    wqk_f = w_in_l[: 2 * D].T.copy()      # [D, 2D]
    wqk_f[:, :D] *= scl_l
    bqk_f = b_in_l[: 2 * D].copy()
    bqk_f[:D] *= scl_l
    # global qk weights: fold 1/64 mean and softmax scale on q side
    wgqk_f = w_in_g[: 2 * D].T.copy()
    wgqk_f[:, :D] *= scl_g / SEG
    wgqk_f[:, D:] *= 1.0 / SEG
    bgqk_f = b_in_g[: 2 * D].copy()
    bgqk_f[:D] *= scl_g
    wgv_f = w_in_g[2 * D:].T.copy() / SEG

    maskl, maskr = _mask_arrays()
    common = {
        "wqk": np.ascontiguousarray(wqk_f).astype(bf),
        "wv": np.ascontiguousarray(w_in_l[2 * D:].T).astype(bf),
        "wo": np.ascontiguousarray(w_out_l.T).astype(bf),
        "wgqk": np.ascontiguousarray(wgqk_f).astype(bf),
        "wgv": np.ascontiguousarray(wgv_f).astype(bf),
        "wgo": np.ascontiguousarray(w_out_g.T).astype(bf),
        "bqk": np.ascontiguousarray(bqk_f.reshape(2 * D, 1)),
        "bgqkr": np.ascontiguousarray(bgqk_f.reshape(1, 2 * D)).astype(bf),
        "bv": np.ascontiguousarray(b_in_l[2 * D:].reshape(1, D)).astype(bf),
        "bgv": np.ascontiguousarray(b_in_g[2 * D:].reshape(1, D)).astype(bf),
        "boc": np.ascontiguousarray(
            (b_out_l + b_out_g).reshape(1, D)
        ).astype(bf),
        "ident": np.eye(128, dtype=f).astype(bf),
        "maskl": maskl,
        "maskr": maskr,
        "bcast": _bcast().astype(bf),
        "ones": np.ones((1, 128), f).astype(bf),
    }
    in_maps = []
    for c in range(N_CORES):
        b, h = divmod(c, 2)
        xT_b = np.ascontiguousarray(x[b].T).astype(bf)  # [D, T]
        loc = xT_b[:, h * TL:(h + 1) * TL]
        rem = xT_b[:, (1 - h) * TL:(2 - h) * TL]
        in_maps.append(
            {
                "xl": np.ascontiguousarray(loc),
                "xr": np.ascontiguousarray(rem),
                **common,
            }
        )
    return in_maps


def _get_runtime():
    """Compile once; return (jitted sharded fn, names metadata)."""
    if "rt" in _CACHE:
        return _CACHE["rt"]
    import jax
    import concourse.mybir as mybir
    from concourse import bass2jax
    from jax.experimental.shard_map import shard_map
    from jax.sharding import Mesh, PartitionSpec

    nc = _build_nc()
    bass2jax.install_neuronx_cc_hook()

    partition_name = nc.partition_id_tensor.name if nc.partition_id_tensor else None
    in_names, out_names, out_avals = [], [], []
    for alloc in nc.m.functions[0].allocations:
        if not isinstance(alloc, mybir.MemoryLocationSet):
            continue
        name = alloc.memorylocations[0].name
        if alloc.kind == "ExternalInput":
            if name != partition_name:
                in_names.append(name)
        elif alloc.kind == "ExternalOutput":
            shape = tuple(alloc.tensor_shape)
            dtype = mybir.dt.np(alloc.dtype)
            out_names.append(name)
            out_avals.append(jax.core.ShapedArray(shape, dtype))
    n_params = len(in_names)
    all_in_names = in_names + out_names
    if partition_name is not None:
        all_in_names = all_in_names + [partition_name]

    def _body(*args):
        operands = list(args)
        if partition_name is not None:
            operands.append(bass2jax.partition_id_tensor())
        outs = bass2jax._bass_exec_p.bind(
            *operands,
            out_avals=tuple(out_avals),
            in_names=tuple(all_in_names),
            out_names=tuple(out_names),
            lowering_input_output_aliases=(),
            sim_require_finite=True,
            sim_require_nnan=True,
            nc=nc,
        )
        return tuple(outs)

    devices = jax.devices()[:N_CORES]
    mesh = Mesh(np.asarray(devices), ("core",))
    in_specs = (PartitionSpec("core"),) * (n_params + len(out_names))
    out_specs = (PartitionSpec("core"),) * len(out_names)
    sharded = jax.jit(
        shard_map(
            _body, mesh=mesh, in_specs=in_specs, out_specs=out_specs, check_rep=False
        ),
        keep_unused=True,
    )
    rt = {
        "nc": nc,
        "sharded": sharded,
        "in_names": in_names,
        "out_names": out_names,
        "out_avals": out_avals,
        "dbg_name": nc.dbg_addr.name if nc.dbg_addr is not None else None,
    }
    _CACHE["rt"] = rt
    return rt


def _concat_args(rt, in_maps):
    """Stack per-core inputs along axis 0 (global view for shard_map)."""
    args = []
    for name in rt["in_names"]:
        if name == rt["dbg_name"]:
            args.append(np.zeros((N_CORES, 2), np.uint32))
            continue
        args.append(np.concatenate([np.asarray(m[name]) for m in in_maps], axis=0))
    for av in rt["out_avals"]:
        args.append(np.zeros((N_CORES * av.shape[0], *av.shape[1:]), av.dtype))
    return args


def _run(in_maps):
    rt = _get_runtime()
    if rt["dbg_name"] is not None:
        for m in in_maps:
            m.setdefault(rt["dbg_name"], np.zeros((1, 2), np.uint32))
    args = _concat_args(rt, in_maps)
    outs = rt["sharded"](*args)
    return [np.asarray(o) for o in outs]


def kernel(**inputs):
    in_maps = _shard_inputs(inputs)
    outs = _run(in_maps)
    out_global = outs[0]  # [8*TL, D]; core c rows [c*TL, (c+1)*TL)
    return out_global.reshape(B, T, D).astype(np.float32)
